# revision 46
# baseline (speedup 1.0000x reference)
"""Enframe kernel for Trainium2 (Bass/Tile), 8-core data parallel.

Problem: input (16, 480000) f32, frame_length=2048, hop=512.
  out[b, w, f] = input[b, w + 512*f],  f in [0, 934), w in [0, 2048).

Key identity: write w = 512*h + l (h in [0,4), l in [0,512)). Then
  out[b, 512*h + l, f] = input[b, 512*(f + h) + l] = in3[b, f + h, l]
where in3 = input[:, :937*512].reshape(B, 937, 512). So the whole op is ONE
(937, 512) -> (512, 937) transpose per clip; the four h-blocks of the output
are shifted overlapping windows T[:, h : h+934] of that transpose.

Shipped default "v11a4" (~51-54 us/iter, vs ~63 us for the old v1Lt):
two-pass permuted store layout ("perm2"), which reaches the 14944 B store
descriptors of the old crashy v8 family without any of its strided-AP ops.

  - Store layout: out row l = 4q + j lives on SBUF partition q with (j, f)
    contiguous per partition, so per-h [128, 4, 934] T2 tiles store with
    14944 B descriptors. Measured: stores-only runs ~36 us (415 GB/s, near
    the 435 GB/s SBUF-fabric rate) vs ~49 us at the v1 layout's 3736 B
    descriptors -- store descriptor size is the dominant lever.
  - Pass 1 (PE, f32): v1's 32 identity transposes per clip into PSUM.
  - Pass 1.5 (DVE): cast copies PSUM -> SBUF bf16 (pass-2 rhs).
  - Pass 2 (PE, bf16): 32 matmuls per clip against a constant host-loaded
    0/1 permutation matrix pmat[m, 4c+j, q] = 1 iff m = 4(q-32c)+j
    (lhsT, accumulated over c with start/stop), routing T row 4q+j onto
    partition q. The permutation matmul itself is exact; the only error is
    the f32->bf16 cast of the data (max rel err 3.9e-3 < the 2e-2 gate).
  - Pass 2.5 (DVE): window copies PSUM -> per-h T2 tiles.
  - Single-ring software pipeline: ALL DMAs ride the sync HWDGE FIFO as
    L(r+1,b0) S(r,b0) L(r+1,b1) S(r,b1); loads run one rep ahead (a_pool
    bufs=4) so no DMA ever sem-waits at the FIFO head.

Why not faster: loads-only ~9 us, stores-only ~36 us, but ANY schedule that
mixes reads+writes (two rings, SWDGE, or single-ring phases) converges to
~52-54 us (~360 GB/s aggregate) -- SDMA engines round-robin logical queues
at packet granularity, so R/W interleave at the memory controller no matter
the issue order. Compute is fully hidden (loads+compute 44.5 us < 52).
Every perm2 variant ran crash-free (~10 fresh processes, >500k HW
executions); the old v8 crash source (strided-free-dim AP pre-permute
copies on ACT/DVE, and strided f32 lhsT transposes = deterministic crash)
is absent from this design by construction.

Measurement notes: machine-level drift is +/-15%, so variants must be
benched with INTERLEAVED rounds (bench_variants.py), never sequentially.
"""

import numpy as np

N_CORES = 8
BATCH = 16
B = BATCH // N_CORES  # clips per core
S = 480000
FRAME = 2048
HOP = 512
F = (S - FRAME) // HOP + 1  # 934
G = FRAME // HOP + F - 1  # 937 distinct 512-sample rows used
G_FULL = G // 128  # 7 full partition chunks
G_TAIL = G - 128 * G_FULL  # 41
H = FRAME // HOP  # 4 output row-blocks of 512

_CACHE: dict = {}


_VARIANTS = {
    # store_mode: "merged" (4 stores/clip, 1.9 MB, p-major enumeration) or
    #             "per_c" (16 stores/clip, 478 KB, sequential DRAM)
    # split_io: cut loads/stores at the psum-half boundary for earlier starts
    "v1": dict(store_mode="merged", split_io=False, bufs=2, psum_bufs=4),
    "v1p": dict(store_mode="merged", split_io=False, bufs=2, psum_bufs=8),
    # split only the loads (not stores): earlier transpose start, same stores
    "v1L": dict(store_mode="merged", split_io=False, split_loads=True, bufs=2, psum_bufs=4),
    # v1L with a 3rd T buffer: decouple copies from store-slot release
    "v1Lt": dict(store_mode="merged", split_io=False, split_loads=True, bufs=2, t_bufs=3, psum_bufs=4),
    # v1Lt with a 4th T buffer
    "v1Lt4": dict(store_mode="merged", split_io=False, split_loads=True, bufs=2, t_bufs=4, psum_bufs=4),
    # v1Lt plus a 3rd A buffer as well
    "v1Lta": dict(store_mode="merged", split_io=False, split_loads=True, bufs=3, t_bufs=3, psum_bufs=4),
    # deep pipeline: 3 A bufs, 4 T bufs, all 8 PSUM banks
    "v1x": dict(store_mode="merged", split_io=False, split_loads=True, bufs=3, t_bufs=4, psum_bufs=8),
    # v1Lt with all 8 PSUM banks
    "v1LtP": dict(store_mode="merged", split_io=False, split_loads=True, bufs=2, t_bufs=3, psum_bufs=8),
    # v1Lt + 3 of 8 stores on the ACT ring (ring byte balance)
    "v6t": dict(store_mode="merged", split_io=False, split_loads=True, bufs=2, t_bufs=3, psum_bufs=4, act_stores=3),
    # half-major production (half outer, c inner) + split stores issued as
    # soon as each half's copies land: halves the T-ready -> store latency
    "v2h": dict(store_mode="merged", split_io=False, split_loads=True, bufs=2, t_bufs=3, psum_bufs=4, half_major=True),
    "v2": dict(store_mode="merged", split_io=True, bufs=2, psum_bufs=8),
    "v3": dict(store_mode="per_c", split_io=False, bufs=2, psum_bufs=4),
    "v4": dict(store_mode="merged", split_io=False, bufs=3, psum_bufs=8),
    "v5": dict(store_mode="per_c", split_io=False, bufs=3, psum_bufs=8),
    # ring balance: n of the 8 stores go to the ACT (scalar) ring alongside
    # the loads, to even out bytes between the two HWDGE rings
    "v6": dict(
        store_mode="merged", split_io=False, bufs=2, psum_bufs=4, act_stores=3
    ),
    "v7": dict(
        store_mode="merged", split_io=False, bufs=2, psum_bufs=4, act_stores=2
    ),
    # timing-only: same DMAs, no transpose/copies — measures the pure DMA
    # ceiling of this access pattern (output is garbage)
    "dma": dict(
        store_mode="merged", split_io=False, bufs=2, psum_bufs=4, dma_only=True
    ),
    # dma-only without loads: pure store ceiling (garbage output)
    "dma_nl": dict(
        store_mode="merged",
        split_io=False,
        bufs=2,
        psum_bufs=4,
        dma_only=True,
        no_loads=True,
    ),
    # loads only, no stores/compute: pure load time (garbage output)
    "dma_ns": dict(
        store_mode="merged",
        split_io=False,
        bufs=2,
        psum_bufs=4,
        dma_only=True,
        store_hs=(),
    ),
    # dma-only with only half the stores: separates bytes-bound from
    # overhead-bound
    "dma2": dict(
        store_mode="merged",
        split_io=False,
        bufs=2,
        psum_bufs=4,
        dma_only=True,
        store_hs=(0, 1),
    ),
    # dma-only, same bytes but idealized stores: 14992 B descriptors into
    # fully linear DRAM — probes whether descriptor size lifts write BW
    "dma3": dict(
        store_mode="linear", split_io=False, bufs=2, psum_bufs=4, dma_only=True
    ),
    # dma-only with the exact v8 store APs (14944 B descriptors, permuted
    # DRAM rows) and memset tiles — isolates the fast-store pattern from the
    # crash-suspect permute path
    "dma4": dict(
        store_mode="interleaved",
        split_io=False,
        bufs=2,
        psum_bufs=4,
        dma_only=True,
    ),
    # dma4 without loads: pure big-descriptor store floor
    "dma4_nl": dict(
        store_mode="interleaved",
        split_io=False,
        bufs=2,
        psum_bufs=4,
        dma_only=True,
        no_loads=True,
    ),
    # interleaved partition mapping: output row l = 4q + j lives on partition
    # q, T tiles are per-h [128, 4, 934] so (j, f) merge into one contiguous
    # 3736-element run -> real 14944 B store descriptors
    # final: interleaved partition mapping with contiguous lhsT via ACT
    # pre-permute. NOTE: adding act_stores or split_io here caused
    # NRT_EXEC_UNIT_UNRECOVERABLE crashes (as "v9") — do not re-add.
    "v8": dict(store_mode="interleaved", split_io=False, bufs=2, psum_bufs=4),
    "v8p": dict(store_mode="interleaved", split_io=False, bufs=2, psum_bufs=8),
    # like v8p but the column pre-permute runs on DVE instead of ACT — the
    # ACT-copy version crashed sporadically (NRT_EXEC_UNIT_UNRECOVERABLE)
    "v8d": dict(
        store_mode="interleaved",
        split_io=False,
        bufs=2,
        psum_bufs=8,
        dve_permute=True,
    ),
    # two-pass permuted store layout: pass1 = v1's transposes, pass1.5 =
    # cast copy psum -> bf16 SBUF, pass2 = bf16 matmul against a constant
    # host-built permutation matrix routing T row 4q+j onto partition q,
    # then v8's 14944 B store APs. No strided-AP ops anywhere (the v8
    # crash suspect). Output is bf16-rounded (rel err ~2e-3 < 2e-2 gate).
    "v10": dict(store_mode="perm2", bufs=2, t_bufs=2, split_loads=True),
    # v10 + SWDGE cast loads (f32 DRAM -> bf16 SBUF on gpsimd): pass1
    # transposes run at bf16 PE rate, halving PE time per rep
    "v10g": dict(
        store_mode="perm2", bufs=2, t_bufs=2, split_loads=True, gp_loads=True
    ),
    # v10 with pass1.5 cast copies on DVE instead of ACT
    "v10d": dict(
        store_mode="perm2",
        bufs=2,
        t_bufs=2,
        split_loads=True,
        p15_dve=True,
    ),
    # v10d with loads on the SYNC ring (same FIFO as stores): serializes
    # read and write phases instead of concurrent R/W mixing
    "v10s": dict(
        store_mode="perm2",
        bufs=2,
        t_bufs=2,
        split_loads=True,
        p15_dve=True,
        sync_loads=True,
    ),
    # v10d + single-ring software-pipelined schedule: all DMAs on the sync
    # FIFO as L(r+1,b0) S(r,b0) L(r+1,b1) S(r,b1) — loads run a rep ahead,
    # giving alternating R/W phases with no FIFO head-of-line blocking
    "v11": dict(
        store_mode="perm2",
        bufs=3,
        t_bufs=2,
        p15_dve=True,
        pipe_loads=True,
    ),
    # v11 with 4 a-buffers (deeper load look-ahead slack)
    "v11a4": dict(
        store_mode="perm2",
        bufs=4,
        t_bufs=2,
        p15_dve=True,
        pipe_loads=True,
    ),
    # v11 with both loads grouped before the stores: 2 instead of 4 ring
    # turnarounds per rep
    "v11r": dict(
        store_mode="perm2",
        bufs=3,
        t_bufs=2,
        p15_dve=True,
        pipe_loads=True,
        pipe_rep_order=True,
    ),
    # v11 + full-g pass2 psum tiles: 16 window copies per clip instead of
    # 32, all on DVE
    "v12": dict(
        store_mode="perm2",
        bufs=3,
        t_bufs=2,
        p15_dve=True,
        pipe_loads=True,
        p2_fullg=True,
        p2_bufs=2,
    ),
    # v12 with the h=3 window copies offloaded to ACT
    "v12a": dict(
        store_mode="perm2",
        bufs=3,
        t_bufs=2,
        p15_dve=True,
        pipe_loads=True,
        p2_fullg=True,
        p2_bufs=2,
        p25_acts=(3,),
    ),
    # v12 with h=2,3 on ACT
    "v12b": dict(
        store_mode="perm2",
        bufs=3,
        t_bufs=2,
        p15_dve=True,
        pipe_loads=True,
        p2_fullg=True,
        p2_bufs=2,
        p25_acts=(2, 3),
    ),
    # v14: v12 + bf16 pass1 via GPSIMD cast (sync-ring f32 loads into a
    # staging tile, idle GpSimd casts to bf16 A) — halves PE pass1 time
    # while keeping the phased single-ring DMA schedule
    "v14": dict(
        store_mode="perm2",
        bufs=2,
        t_bufs=2,
        p15_dve=True,
        pipe_loads=True,
        p2_fullg=True,
        p2_bufs=2,
        gp_cast=True,
    ),
    # v11a4 with BOTH clips' loads fused into one DMA pair per rep: a
    # single uninterrupted read phase per rep, 2 read-FIFO slots instead
    # of 4-6
    "v17": dict(
        store_mode="perm2",
        bufs=2,
        t_bufs=2,
        p15_dve=True,
        pipe_loads=True,
        fused_loads=True,
    ),
    # v11a4 + single_packet DMAs: each engine drains a whole DMA's packet
    # before the queue round-robin switches, coarsening R/W interleave to
    # whole-DMA granularity (all our DMAs fit the 64-desc packet ceiling)
    "v15": dict(
        store_mode="perm2",
        bufs=4,
        t_bufs=2,
        p15_dve=True,
        pipe_loads=True,
        sp_dma=True,
    ),
    # v15 with single_packet only on the stores
    "v15s": dict(
        store_mode="perm2",
        bufs=4,
        t_bufs=2,
        p15_dve=True,
        pipe_loads=True,
        sp_dma_stores=True,
    ),
    # v14 with the cast on DVE instead of GPSIMD (fallback / comparison)
    "v14d": dict(
        store_mode="perm2",
        bufs=2,
        t_bufs=2,
        p15_dve=True,
        pipe_loads=True,
        p2_fullg=True,
        p2_bufs=2,
        dve_cast=True,
    ),
    # v12 without load pipelining (concurrent loads on scalar ring)
    "v12c": dict(
        store_mode="perm2",
        bufs=2,
        t_bufs=2,
        split_loads=True,
        p15_dve=True,
        p2_fullg=True,
        p2_bufs=2,
    ),
    # probe: v10d without stores — times the load+compute pipeline alone
    "v10cnl": dict(
        store_mode="perm2",
        bufs=2,
        t_bufs=2,
        split_loads=True,
        p15_dve=True,
        no_stores=True,
    ),
    # probe: v10d without loads (garbage data) — compute+store pipeline
    "v10cns": dict(
        store_mode="perm2",
        bufs=2,
        t_bufs=2,
        p15_dve=True,
        no_loads=True,
    ),
    # v10d with ONE merged store per clip (descriptors unchanged at
    # 14944 B; 2 instead of 8 store DMAs per rep)
    "v10m": dict(
        store_mode="perm2",
        bufs=2,
        t_bufs=2,
        split_loads=True,
        p15_dve=True,
        merged_store=True,
    ),
    # v10g with pass1.5 on DVE
    "v10gd": dict(
        store_mode="perm2",
        bufs=2,
        t_bufs=2,
        split_loads=True,
        gp_loads=True,
        p15_dve=True,
    ),
}


def _perm2_pmat() -> np.ndarray:
    """pmat[m, 4c+j, q] = 1 iff m = 4(q-32c)+j for q in [32c, 32(c+1)).

    Used as matmul lhsT: out[q, g] = sum_m pmat[m, 4c+j, q] * Sb_c[m, g]
    accumulated over c gives out[q, g] = T[4q+j, g]."""
    import ml_dtypes

    pm = np.zeros((128, 16, 128), np.float32)
    for c in range(4):
        for j in range(4):
            for t in range(32):
                pm[4 * t + j, 4 * c + j, 32 * c + t] = 1.0
    return pm.astype(ml_dtypes.bfloat16)


def _variant_extras(variant: str) -> list[np.ndarray]:
    """Extra DRAM parameters (beyond "input"), tiled across the 8 cores."""
    if _VARIANTS[variant].get("store_mode") != "perm2":
        return []
    pm = _perm2_pmat()
    return [
        np.ascontiguousarray(
            np.broadcast_to(pm, (N_CORES, *pm.shape)).reshape(
                N_CORES * 128, 16, 128
            )
        )
    ]


def _build_program(reps: int, variant: str = "v1Lt"):
    from concourse import bass, masks, mybir
    from concourse.tile import TileContext

    cfg = _VARIANTS[variant]
    split_io = cfg.get("split_io", False)
    store_mode = cfg["store_mode"]
    bufs = cfg["bufs"]
    psum_bufs = cfg.get("psum_bufs", 4)
    act_stores = cfg.get("act_stores", 0)
    # spread the ACT-ring stores evenly over the 8 (b, h) store slots
    act_slots = set()
    if act_stores:
        stride = (B * H) / act_stores
        act_slots = {int(i * stride + stride / 2) for i in range(act_stores)}

    F32 = mybir.dt.float32
    BF16 = mybir.dt.bfloat16
    perm2 = store_mode == "perm2"
    nc = bass.Bass()
    inp = nc.declare_dram_parameter("input", [B, S], F32, isOutput=False)
    pmat_d = None
    if perm2:
        pmat_d = nc.declare_dram_parameter(
            "pmat", [128, 16, 128], BF16, isOutput=False
        )
    outp = nc.declare_dram_parameter("out", [B, FRAME, F], F32, isOutput=True)

    with TileContext(nc) as tc:
        with (
            tc.tile_pool(name="ident_pool", bufs=1) as ipool,
            tc.tile_pool(name="a_pool", bufs=bufs) as apool,
            tc.tile_pool(name="t_pool", bufs=cfg.get("t_bufs", bufs)) as tpool,
            tc.tile_pool(name="psum_pool", bufs=psum_bufs, space="PSUM") as ppool,
            tc.tile_pool(name="sb_pool", bufs=16) as sbpool,
            tc.tile_pool(
                name="p2_pool", bufs=cfg.get("p2_bufs", 4), space="PSUM"
            ) as p2pool,
        ):
            adt = (
                BF16
                if (
                    cfg.get("gp_loads")
                    or cfg.get("gp_cast")
                    or cfg.get("dve_cast")
                )
                else F32
            )
            ident = ipool.tile([128, 128], adt)
            masks.make_identity(nc, ident[:])
            pmat_sb = None
            if perm2:
                pmat_sb = ipool.tile([128, 16, 128], BF16, tag="pmat")
                nc.scalar.dma_start(out=pmat_sb[:, :, :], in_=pmat_d[:, :, :])

            def p2_clip(b, a_t):
                """perm2 body for one clip: pass1 transposes, pass1.5 cast
                copies, pass2 permute matmuls, window copies, stores."""
                sbs = [[None, None] for _ in range(4)]
                fused = cfg.get("fused_loads")
                for half in range(2):
                    glen = 512 if half == 0 else G - 512  # 425
                    for c in range(4):
                        ps1 = ppool.tile(
                            [128, 512], adt, tag="ps", name="ps1"
                        )
                        for k in range(4):
                            h8 = 4 * half + k
                            rows = 128 if h8 < G_FULL else G_TAIL
                            src = (
                                a_t[0:rows, b, h8, 128 * c : 128 * (c + 1)]
                                if fused
                                else a_t[0:rows, h8, 128 * c : 128 * (c + 1)]
                            )
                            nc.tensor.transpose(
                                out=ps1[:, 128 * k : 128 * k + rows],
                                in_=src,
                                identity=ident[0:rows, 0:rows],
                            )
                        sb = sbpool.tile([128, 512], BF16, tag="sb", name="sb")
                        if cfg.get("p15_dve"):
                            nc.vector.tensor_copy(
                                out=sb[:, 0:glen], in_=ps1[:, 0:glen]
                            )
                        else:
                            nc.scalar.copy(
                                out=sb[:, 0:glen], in_=ps1[:, 0:glen]
                            )
                        sbs[c][half] = sb
                t2 = [
                    tpool.tile([128, 4, F], F32, tag=f"t2_{h}", name=f"t2_{h}")
                    for h in range(H)
                ]
                if cfg.get("p2_fullg"):
                    # pass2 into full-g psum tiles [128, 937] (2 banks):
                    # one window copy per (j, h) instead of two partials
                    p25_acts = cfg.get("p25_acts", ())
                    for j in range(4):
                        p2 = p2pool.tile([128, G], F32, tag="p2g", name="p2g")
                        for half in range(2):
                            glen = 512 if half == 0 else G - 512
                            for c in range(4):
                                nc.tensor.matmul(
                                    p2[:, 512 * half : 512 * half + glen],
                                    pmat_sb[:, 4 * c + j, :],
                                    sbs[c][half][:, 0:glen],
                                    start=(c == 0),
                                    stop=(c == 3),
                                )
                        # p2[q, g] = T[4q+j, g]
                        for h in range(H):
                            if h in p25_acts:
                                nc.scalar.copy(
                                    out=t2[h][:, j, :], in_=p2[:, h : h + F]
                                )
                            else:
                                nc.vector.tensor_copy(
                                    out=t2[h][:, j, :], in_=p2[:, h : h + F]
                                )
                else:
                    for half in range(2):
                        glen = 512 if half == 0 else G - 512
                        for j in range(4):
                            p2 = p2pool.tile(
                                [128, 512], F32, tag="p2", name="p2"
                            )
                            for c in range(4):
                                nc.tensor.matmul(
                                    p2[:, 0:glen],
                                    pmat_sb[:, 4 * c + j, :],
                                    sbs[c][half][:, 0:glen],
                                    start=(c == 0),
                                    stop=(c == 3),
                                )
                            # p2[q, col] = T row (4q+j), g = 512*half+col
                            for h in range(H):
                                if half == 0:
                                    nc.vector.tensor_copy(
                                        out=t2[h][:, j, 0 : 512 - h],
                                        in_=p2[:, h:512],
                                    )
                                else:
                                    ln = 422 + h
                                    nc.vector.tensor_copy(
                                        out=t2[h][:, j, 512 - h : 512 - h + ln],
                                        in_=p2[:, 0:ln],
                                    )
                if not cfg.get("no_stores"):
                    sp = bool(
                        cfg.get("sp_dma") or cfg.get("sp_dma_stores")
                    )
                    for h in range(H):
                        nc.sync.dma_start(
                            out=outp[b, 512 * h : 512 * (h + 1), :].rearrange(
                                "(q j) f -> q (j f)", q=128, j=4
                            ),
                            in_=t2[h][:, :, :].rearrange("p j f -> p (j f)"),
                            single_packet=sp,
                        )

            if perm2 and cfg.get("pipe_loads"):
                # Software-pipelined single-ring schedule: ALL DMAs ride the
                # sync HWDGE FIFO in the order L(r+1,b0) S(r,b0) L(r+1,b1)
                # S(r,b1). Loads run one rep ahead, so every DMA's sems are
                # satisfied before its FIFO turn and the ring alternates
                # clean read/write phases (no concurrent R/W mixing, no
                # head-of-line blocking).
                def p2_load(b):
                    on_chip_cast = cfg.get("gp_cast") or cfg.get("dve_cast")
                    ldt = F32 if on_chip_cast else adt
                    tag = "a_st" if on_chip_cast else "a"
                    stage = apool.tile(
                        [128, G_FULL + 1, HOP], ldt, tag=tag, name="a_ld"
                    )
                    sp_ld = bool(cfg.get("sp_dma"))
                    nc.sync.dma_start(
                        out=stage[:, 0:G_FULL, :],
                        in_=inp[b, 0 : 128 * G_FULL * HOP].rearrange(
                            "(h p c) -> p h c", h=G_FULL, p=128, c=HOP
                        ),
                        single_packet=sp_ld,
                    )
                    nc.sync.dma_start(
                        out=stage[0:G_TAIL, G_FULL, :],
                        in_=inp[b, 128 * G_FULL * HOP : G * HOP].rearrange(
                            "(p c) -> p c", p=G_TAIL, c=HOP
                        ),
                        single_packet=sp_ld,
                    )
                    if not on_chip_cast:
                        return stage
                    a_t = apool.tile(
                        [128, G_FULL + 1, HOP], BF16, tag="a", name="a_t"
                    )
                    ce = nc.gpsimd if cfg.get("gp_cast") else nc.vector
                    ce.tensor_copy(
                        out=a_t[:, 0:G_FULL, :], in_=stage[:, 0:G_FULL, :]
                    )
                    ce.tensor_copy(
                        out=a_t[0:G_TAIL, G_FULL, :],
                        in_=stage[0:G_TAIL, G_FULL, :],
                    )
                    return a_t

                def p2_load_fused():
                    # one a2 tile per rep; DMA APs are capped at 3 dims, so
                    # mains stay per-clip but tails fuse across clips
                    a2 = apool.tile(
                        [128, B, G_FULL + 1, HOP], F32, tag="a2", name="a2"
                    )
                    for bb in range(B):
                        nc.sync.dma_start(
                            out=a2[:, bb, 0:G_FULL, :],
                            in_=inp[bb, 0 : 128 * G_FULL * HOP].rearrange(
                                "(h p c) -> p h c", h=G_FULL, p=128, c=HOP
                            ),
                        )
                    nc.sync.dma_start(
                        out=a2[0:G_TAIL, :, G_FULL, :],
                        in_=inp[:, 128 * G_FULL * HOP : G * HOP].rearrange(
                            "b (p c) -> p b c", p=G_TAIL, c=HOP
                        ),
                    )
                    return a2

                if cfg.get("fused_loads"):
                    cur_f = p2_load_fused()
                    for r in range(reps):
                        nxt_f = p2_load_fused() if r + 1 < reps else None
                        for b in range(B):
                            p2_clip(b, cur_f)
                        cur_f = nxt_f
                    reps = 0

                cur = [p2_load(b) for b in range(B)] if reps else []
                for r in range(reps):
                    nxt = []
                    if cfg.get("pipe_rep_order"):
                        # both loads first: 2 R/W turnarounds per rep
                        if r + 1 < reps:
                            nxt = [p2_load(b) for b in range(B)]
                        for b in range(B):
                            p2_clip(b, cur[b])
                    else:
                        for b in range(B):
                            if r + 1 < reps:
                                nxt.append(p2_load(b))
                            p2_clip(b, cur[b])
                    cur = nxt
                # skip the legacy rep loop below
                reps = 0

            for _rep in range(reps):
                # loads for both clips upfront (own HWDGE ring via nc.scalar):
                # split at the h8=4 boundary so half-0 transposes start after
                # the first MB.
                a_ts = []
                ld = nc.gpsimd if cfg.get("gp_loads") else nc.scalar
                if cfg.get("sync_loads"):
                    ld = nc.sync
                for b in range(B):
                    a_t = apool.tile([128, G_FULL + 1, HOP], adt, tag="a")
                    a_ts.append(a_t)
                    if cfg.get("no_loads"):
                        # probe: give a_t a writer so Tile allocates it
                        nc.vector.memset(a_t[:, 0, 0:1], 0.0)
                        continue
                    # rows g = h8*128 + p hold samples 512g .. 512g+512
                    if split_io or cfg.get("split_loads"):
                        ld.dma_start(
                            out=a_t[:, 0:4, :],
                            in_=inp[b, 0 : 128 * 4 * HOP].rearrange(
                                "(h p c) -> p h c", h=4, p=128, c=HOP
                            ),
                        )
                        ld.dma_start(
                            out=a_t[:, 4:G_FULL, :],
                            in_=inp[
                                b, 128 * 4 * HOP : 128 * G_FULL * HOP
                            ].rearrange(
                                "(h p c) -> p h c", h=G_FULL - 4, p=128, c=HOP
                            ),
                        )
                    else:
                        ld.dma_start(
                            out=a_t[:, 0:G_FULL, :],
                            in_=inp[b, 0 : 128 * G_FULL * HOP].rearrange(
                                "(h p c) -> p h c", h=G_FULL, p=128, c=HOP
                            ),
                        )
                    # tail: last 41 rows
                    ld.dma_start(
                        out=a_t[0:G_TAIL, G_FULL, :],
                        in_=inp[b, 128 * G_FULL * HOP : G * HOP].rearrange(
                            "(p c) -> p c", p=G_TAIL, c=HOP
                        ),
                    )

                for b in range(B):
                    a_t = a_ts[b]
                    if store_mode == "perm2" and not cfg.get("merged_store"):
                        p2_clip(b, a_t)
                        continue
                    if store_mode == "perm2":
                        # pass 1: v1-style transposes, half-major order so
                        # pass-2 half-0 groups can start early
                        sbs = [[None, None] for _ in range(4)]
                        for half in range(2):
                            glen = 512 if half == 0 else G - 512  # 425
                            for c in range(4):
                                ps1 = ppool.tile([128, 512], adt, tag="ps")
                                for k in range(4):
                                    h8 = 4 * half + k
                                    rows = 128 if h8 < G_FULL else G_TAIL
                                    nc.tensor.transpose(
                                        out=ps1[:, 128 * k : 128 * k + rows],
                                        in_=a_t[
                                            0:rows, h8, 128 * c : 128 * (c + 1)
                                        ],
                                        identity=ident[0:rows, 0:rows],
                                    )
                                # pass 1.5: psum -> SBUF bf16 for pass-2 rhs
                                sb = sbpool.tile([128, 512], BF16, tag="sb")
                                if cfg.get("p15_dve"):
                                    nc.vector.tensor_copy(
                                        out=sb[:, 0:glen], in_=ps1[:, 0:glen]
                                    )
                                else:
                                    nc.scalar.copy(
                                        out=sb[:, 0:glen], in_=ps1[:, 0:glen]
                                    )
                                sbs[c][half] = sb
                        merged = cfg.get("merged_store")
                        if merged:
                            t2all = tpool.tile(
                                [128, H, 4, F], F32, tag="t2all", name="t2all"
                            )
                            t2 = None
                        else:
                            t2all = None
                            t2 = [
                                tpool.tile(
                                    [128, 4, F],
                                    F32,
                                    tag=f"t2_{h}",
                                    name=f"t2_{h}",
                                )
                                for h in range(H)
                            ]
                        # pass 2: route T row 4q+j onto partition q via the
                        # constant permutation lhsT, accumulating over c
                        for half in range(2):
                            glen = 512 if half == 0 else G - 512
                            for j in range(4):
                                p2 = p2pool.tile([128, 512], F32, tag="p2")
                                for c in range(4):
                                    nc.tensor.matmul(
                                        p2[:, 0:glen],
                                        pmat_sb[:, 4 * c + j, :],
                                        sbs[c][half][:, 0:glen],
                                        start=(c == 0),
                                        stop=(c == 3),
                                    )
                                # p2[q, col] = T row (4q+j), g = 512*half+col
                                for h in range(H):
                                    if half == 0:
                                        sl = slice(0, 512 - h)
                                        src = p2[:, h:512]
                                    else:
                                        ln = 422 + h
                                        sl = slice(512 - h, 512 - h + ln)
                                        src = p2[:, 0:ln]
                                    dst = (
                                        t2all[:, h, j, sl]
                                        if merged
                                        else t2[h][:, j, sl]
                                    )
                                    nc.vector.tensor_copy(out=dst, in_=src)
                        if merged:
                            nc.sync.dma_start(
                                out=outp[b].rearrange(
                                    "(h q j) f -> q h (j f)", h=H, q=128, j=4
                                ),
                                in_=t2all[:, :, :, :].rearrange(
                                    "p h j f -> p h (j f)"
                                ),
                            )
                        elif not cfg.get("no_stores"):
                            for h in range(H):
                                nc.sync.dma_start(
                                    out=outp[
                                        b, 512 * h : 512 * (h + 1), :
                                    ].rearrange("(q j) f -> q (j f)", q=128, j=4),
                                    in_=t2[h][:, :, :].rearrange(
                                        "p j f -> p (j f)"
                                    ),
                                )
                        continue
                    if store_mode == "interleaved":
                        # T2h[q, j, f] = out[b, 512h + 4q + j, f]; per-h tiles
                        # of exactly [128, 4, 934] make (j, f) contiguous per
                        # partition -> 14944 B store descriptors.
                        if cfg.get("dma_only"):
                            # probe: v8 store APs with memset data — times the
                            # 14944 B store pattern without the permute path
                            t2 = [
                                tpool.tile(
                                    [128, 4, F],
                                    F32,
                                    tag=f"t2_{h}",
                                    name=f"t2_{h}",
                                )
                                for h in range(H)
                            ]
                            for h in range(H):
                                nc.vector.memset(t2[h][:, 0, 0:1], 0.0)
                                nc.sync.dma_start(
                                    out=outp[
                                        b, 512 * h : 512 * (h + 1), :
                                    ].rearrange("(q j) f -> q (j f)", q=128, j=4),
                                    in_=t2[h][:, :, :].rearrange(
                                        "p j f -> p (j f)"
                                    ),
                                )
                            continue
                        #
                        # A strided-free-dim f32 lhsT crashes the NC
                        # (NRT_EXEC_UNIT_UNRECOVERABLE, probed in isolation),
                        # so pre-permute columns on ACT: a_perm[p, h8, j, q] =
                        # a_t[p, h8, 4q + j]; every matmul then reads a
                        # contiguous 128-column slice.
                        a_perm = apool.tile(
                            [128, G_FULL + 1, 4, 128], F32, tag="a_perm"
                        )
                        perm_copy = (
                            nc.vector.tensor_copy
                            if cfg.get("dve_permute")
                            else nc.scalar.copy
                        )
                        perm_copy(
                            out=a_perm[:, 0:G_FULL, :, :],
                            in_=a_t[:, 0:G_FULL, :].rearrange(
                                "p h (q j) -> p h j q", q=128, j=4
                            ),
                        )
                        perm_copy(
                            out=a_perm[0:G_TAIL, G_FULL, :, :],
                            in_=a_t[0:G_TAIL, G_FULL, :].rearrange(
                                "p (q j) -> p j q", q=128, j=4
                            ),
                        )
                        t2 = [
                            tpool.tile(
                                [128, 4, F], F32, tag=f"t2_{h}", name=f"t2_{h}"
                            )
                            for h in range(H)
                        ]
                        for j in range(4):
                            for half in range(2):
                                ps = ppool.tile([128, 512], F32, tag="ps")
                                glen = 512 if half == 0 else G - 512  # 425
                                for k in range(4):
                                    h8 = 4 * half + k
                                    rows = 128 if h8 < G_FULL else G_TAIL
                                    nc.tensor.transpose(
                                        out=ps[:, 128 * k : 128 * k + rows],
                                        in_=a_perm[0:rows, h8, j, :],
                                        identity=ident[0:rows, 0:rows],
                                    )
                                # ps[q, col] = T row (4q+j), g = 512*half+col
                                for h in range(H):
                                    if half == 0:
                                        # f in [0, 512-h) <- g = h + f
                                        nc.vector.tensor_copy(
                                            out=t2[h][:, j, 0 : 512 - h],
                                            in_=ps[:, h:512],
                                        )
                                    else:
                                        # f in [512-h, ...) <- g = h + f
                                        ln = min(422 + h, glen)
                                        nc.vector.tensor_copy(
                                            out=t2[h][:, j, 512 - h : 512 - h + ln],
                                            in_=ps[:, 0:ln],
                                        )
                        for h in range(H):
                            eng = (
                                nc.scalar
                                if (b * H + h) in act_slots
                                else nc.sync
                            )
                            eng.dma_start(
                                out=outp[b, 512 * h : 512 * (h + 1), :].rearrange(
                                    "(q j) f -> q (j f)", q=128, j=4
                                ),
                                in_=t2[h][:, :, :].rearrange("p j f -> p (j f)"),
                            )
                        continue
                    t_t = tpool.tile([128, 4, G], F32, tag="t")
                    if cfg.get("dma_only"):
                        # give t_t a writer so Tile allocates it
                        nc.vector.memset(t_t[:, 0, 0:1], 0.0)
                    if cfg.get("half_major"):
                        # produce half 0 for all c, store its split slice
                        # immediately, then half 1 + the rest
                        for half in range(2):
                            glen = 512 if half == 0 else G - 512  # 425
                            for c in range(4):
                                ps = ppool.tile([128, 512], F32, tag="ps")
                                for k in range(4):
                                    h8 = 4 * half + k
                                    rows = 128 if h8 < G_FULL else G_TAIL
                                    nc.tensor.transpose(
                                        out=ps[:, 128 * k : 128 * k + rows],
                                        in_=a_t[
                                            0:rows, h8, 128 * c : 128 * (c + 1)
                                        ],
                                        identity=ident[0:rows, 0:rows],
                                    )
                                nc.vector.tensor_copy(
                                    out=t_t[:, c, 512 * half : 512 * half + glen],
                                    in_=ps[:, 0:glen],
                                )
                            for h in range(H):
                                dram = outp[
                                    b, 512 * h : 512 * (h + 1), :
                                ].rearrange("(c p) f -> p c f", c=4, p=128)
                                fsplit = 512 - h
                                if half == 0:
                                    nc.sync.dma_start(
                                        out=dram[:, :, 0:fsplit],
                                        in_=t_t[:, :, h:512],
                                    )
                                else:
                                    nc.sync.dma_start(
                                        out=dram[:, :, fsplit:F],
                                        in_=t_t[:, :, 512 : h + F],
                                    )
                        continue
                    for c in range(4):
                        if cfg.get("dma_only"):
                            break
                        for half in range(2):
                            ps = ppool.tile([128, 512], F32, tag="ps")
                            glen = 512 if half == 0 else G - 512  # 425
                            for k in range(4):
                                h8 = 4 * half + k
                                rows = 128 if h8 < G_FULL else G_TAIL
                                nc.tensor.transpose(
                                    out=ps[:, 128 * k : 128 * k + rows],
                                    in_=a_t[0:rows, h8, 128 * c : 128 * (c + 1)],
                                    identity=ident[0:rows, 0:rows],
                                )
                            nc.vector.tensor_copy(
                                out=t_t[:, c, 512 * half : 512 * half + glen],
                                in_=ps[:, 0:glen],
                            )

                    if store_mode == "linear":
                        # timing-only: 4 stores x [128, 3748] covering the
                        # same output bytes with 14992 B linear descriptors
                        flat = outp[b].rearrange("w f -> (w f)")
                        n = 128 * 3736
                        for i in range(4):
                            nc.sync.dma_start(
                                out=flat[i * n : (i + 1) * n].rearrange(
                                    "(p q) -> p q", p=128, q=3736
                                ),
                                in_=t_t[:, :, :].rearrange("p c g -> p (c g)")[
                                    :, 0:3736
                                ],
                            )
                        continue
                    for h in cfg.get("store_hs", range(H)):
                        # DRAM rows 512*h + c*128 + p; descriptors are
                        # contiguous 3736 B f-runs either way.
                        if store_mode == "per_c":
                            # one store per c-block: [128, 934], DRAM fully
                            # sequential within the store
                            for c in range(4):
                                nc.sync.dma_start(
                                    out=outp[
                                        b,
                                        512 * h + 128 * c : 512 * h + 128 * (c + 1),
                                        :,
                                    ],
                                    in_=t_t[:, c, h : h + F],
                                )
                            continue
                        dram = outp[b, 512 * h : 512 * (h + 1), :].rearrange(
                            "(c p) f -> p c f", c=4, p=128
                        )
                        if split_io:
                            fsplit = 512 - h
                            nc.sync.dma_start(
                                out=dram[:, :, 0:fsplit],
                                in_=t_t[:, :, h : h + fsplit],
                            )
                            nc.sync.dma_start(
                                out=dram[:, :, fsplit:F],
                                in_=t_t[:, :, 512 : h + F],
                            )
                        else:
                            eng = (
                                nc.scalar
                                if (b * H + h) in act_slots
                                else nc.sync
                            )
                            eng.dma_start(
                                out=dram, in_=t_t[:, :, h : h + F]
                            )

    # TRN2 Matmult (and most instructions) encode at most 1 sync wait; the
    # Tile flow skips the bacc pass that splits extra waits into
    # InstEventSemaphore carriers, so run it here.
    import bass_rust

    bass_rust.generate_event_semaphores(nc)
    return nc


class _Runner:
    """Persistent jitted SPMD runner (modeled on bass2jax.run_bass_via_pjrt,
    but caches the jitted executable across calls).

    donate=False keeps the zero output-donor buffers reusable across calls,
    which lets timing loops run with fully device-resident operands."""

    def __init__(self, reps: int, donate: bool = True, variant: str = "v1Lt"):
        import jax
        from concourse import bass2jax, mybir
        from jax.experimental.shard_map import shard_map
        from jax.sharding import Mesh, PartitionSpec

        bass2jax.install_neuronx_cc_hook()
        self._jax = jax
        nc = _build_program(reps, variant)
        self._nc = nc
        self._extras = _variant_extras(variant)

        partition_name = (
            nc.partition_id_tensor.name if nc.partition_id_tensor else None
        )
        in_names: list[str] = []
        out_names: list[str] = []
        out_avals = []
        self._zero_shapes = []
        for alloc in nc.m.functions[0].allocations:
            if not isinstance(alloc, mybir.MemoryLocationSet):
                continue
            name = alloc.memorylocations[0].name
            if alloc.kind == "ExternalInput":
                if name != partition_name:
                    in_names.append(name)
            elif alloc.kind == "ExternalOutput":
                out_names.append(name)
                shape = tuple(alloc.tensor_shape)
                dtype = mybir.dt.np(alloc.dtype)
                out_avals.append(jax.core.ShapedArray(shape, dtype))
                self._zero_shapes.append((shape, dtype))
        n_params = len(in_names)
        n_outs = len(out_avals)
        assert n_params == 1 + len(self._extras), (in_names, len(self._extras))
        in_names_full = [*in_names, *out_names]
        if partition_name is not None:
            in_names_full.append(partition_name)

        def _body(*args):
            operands = list(args)
            if partition_name is not None:
                operands.append(bass2jax.partition_id_tensor())
            outs = bass2jax._bass_exec_p.bind(
                *operands,
                out_avals=tuple(out_avals),
                in_names=tuple(in_names_full),
                out_names=tuple(out_names),
                lowering_input_output_aliases=(),
                sim_require_finite=True,
                sim_require_nnan=True,
                nc=nc,
            )
            return tuple(outs)

        devices = jax.devices()[:N_CORES]
        assert len(devices) == N_CORES, devices
        mesh = Mesh(np.asarray(devices), ("core",))
        self._mesh = mesh
        self._pspec = PartitionSpec("core")
        donate_argnums = (
            tuple(range(n_params, n_params + n_outs)) if donate else ()
        )
        self._sharded = jax.jit(
            shard_map(
                _body,
                mesh=mesh,
                in_specs=(PartitionSpec("core"),) * (n_params + n_outs),
                out_specs=(PartitionSpec("core"),) * n_outs,
                check_rep=False,
            ),
            donate_argnums=donate_argnums,
            keep_unused=True,
        )

    def fresh_zeros(self):
        return [
            np.zeros((N_CORES * s[0], *s[1:]), d) for s, d in self._zero_shapes
        ]

    def __call__(self, x: np.ndarray, zeros=None):
        # shard_map splits axis 0 across the 8 cores: rows [2i, 2i+2) land on
        # core i — exactly the batch sharding. Global in/out pass through.
        if zeros is None:
            zeros = self.fresh_zeros()
        out = self._sharded(x, *self._extras, *zeros)[0]
        return np.asarray(out)

    def device_args(self, x: np.ndarray):
        """device_put the operands once, sharded over the mesh."""
        import jax
        from jax.sharding import NamedSharding

        sh = NamedSharding(self._mesh, self._pspec)
        return [
            jax.device_put(a, sh)
            for a in (x, *self._extras, *self.fresh_zeros())
        ]

    def dispatch(self, args):
        """Launch without fetching results; returns device array handles."""
        return self._sharded(*args)


def get_runner(reps: int = 1, donate: bool = True, variant: str = "v11a4") -> "_Runner":
    key = ("runner", reps, donate, variant)
    if key not in _CACHE:
        _CACHE[key] = _Runner(reps, donate, variant)
    return _CACHE[key]


def kernel(input: np.ndarray) -> np.ndarray:
    x = np.ascontiguousarray(input, dtype=np.float32)
    assert x.shape == (BATCH, S), x.shape
    return get_runner(1)(x)

